# revision 23
# baseline (speedup 1.0000x reference)
"""Trainium2 Bass kernel for nn_Decoder (LSTMCell -> GRUCell -> Linear decode).

Strategy (8 NeuronCores, one chip):
  - Hidden dim H=2048 sharded 8 ways (256/core). Each core holds the weight
    rows for its hidden slice of the LSTM/GRU gates in SBUF (bf16), computes
    its gate shard with batch-major col-tiled matmuls (stationary = h-major
    state tiles [128,32], moving = weight columns), applies the elementwise
    cell updates in fp32, stream-transposes its new state shard to h-major
    bf16 and exchanges it via two pipelined AllGathers per step: AG_hc
    carries [h(s+1), c(s+1)] and AG_hg carries [hg(s)].
  - Pipeline: the GRU lags the LSTM by one window, and in window s the
    GRU(s-1) (whose inputs were gathered a window earlier) is EMITTED
    BEFORE the h-gated LSTM(s+1) - engine queues are FIFO, so this lets
    the GRU fill TensorE while AG_hc[s] is in flight. Queue assignment
    keeps gathers off congested FIFOs (state gathers on sync, hg bounce
    on gpsimd, decode writeback on scalar). LSTM gates are hosted in
    (i,f,o,g) order so one fused sigmoid covers i|f|o.
  - Hidden-index layout l = 128*c2 + 32*jp + i is chosen so the 32x32-block
    StreamTranspose of the [128,64] state tile directly yields the h-major
    shard, and every DRAM exchange is a single strided DMA.
  - The vocab-sharded linear decode (1000 rows/core) consumes a double-
    buffered history of gathered hg, one vocab tile per step, hiding under
    the collective latency and keeping TensorE warm.
  - kernel(**inputs) takes FULL inputs, shards on host, runs the SPMD NEFF
    on cores 0-7 via run_bass_kernel_spmd, reassembles the FULL output.
"""
import os
import sys

import numpy as np

for _p in ("/root/.axon_site", "/root/.axon_site/_ro/trn_rl_repo",
           "/root/.axon_site/_ro/pypackages", "/opt/trn_rl_repo"):
    if os.path.isdir(_p) and _p not in sys.path:
        sys.path.append(_p)

import concourse.bacc as bacc
import concourse.bass as bass
import concourse.mybir as mybir
import concourse.tile as tile
from concourse import bass_utils

import ml_dtypes

BF16 = ml_dtypes.bfloat16
F32 = mybir.dt.float32
BF = mybir.dt.bfloat16
AF = mybir.ActivationFunctionType

NC = 8          # cores
B = 32          # batch
T = 40          # caption length
TS = T - 1      # recurrent steps
V = 8000
E = 50
H = 2048
HS = H // NC    # 256 hidden per core
VS = V // NC    # 1000 vocab per core
KT = H // 128   # 16 contraction tiles
VT = 8          # vocab tiles per core
VTW = VS // VT  # 125 cols per vocab tile
GL = 4 * HS     # 1024 lstm gate cols per core
GG = 3 * HS     # 768 gru gate cols per matmul per core
NHIST = 8       # decode chunk length

_BUILD_CACHE = {}


def _build(ts=TS):
    nc = bacc.Bacc("TRN2", target_bir_lowering=False, debug=False,
                   enable_asserts=True, num_devices=NC)

    # ---- external I/O (per core) ----
    wl_in = nc.dram_tensor("wl", [H, GL], BF, kind="ExternalInput")
    wih_in = nc.dram_tensor("wih", [E + 1, GL], BF, kind="ExternalInput")
    wgi_in = nc.dram_tensor("wgi", [H, GG], BF, kind="ExternalInput")
    bgi_in = nc.dram_tensor("bgi", [128, GG // 4], BF, kind="ExternalInput")
    wgh_in = nc.dram_tensor("wgh", [H, GG], BF, kind="ExternalInput")
    bgh_in = nc.dram_tensor("bgh", [1, GG], BF, kind="ExternalInput")
    linw_in = nc.dram_tensor("linw", [H, VS], BF, kind="ExternalInput")
    linb_in = nc.dram_tensor("linb", [VTW, VT], F32, kind="ExternalInput")
    xs_in = nc.dram_tensor("xs_aug", [E + 1, ts * B], BF, kind="ExternalInput")
    featT_in = nc.dram_tensor("featT", [H, B], BF, kind="ExternalInput")
    featsh_in = nc.dram_tensor("feat_sh", [HS, B], BF, kind="ExternalInput")
    featblk_in = nc.dram_tensor("feat_blk", [128, 64], F32, kind="ExternalInput")
    ones_in = nc.dram_tensor("ones", [1, B], BF, kind="ExternalInput")
    out = nc.dram_tensor("out", [VS, ts * B], F32, kind="ExternalOutput")

    with tile.TileContext(nc) as tc:
        with (
            tc.tile_pool(name="const", bufs=1) as cpool,
            tc.tile_pool(name="stat", bufs=3) as spool,
            tc.tile_pool(name="state", bufs=2) as stpool,
            tc.tile_pool(name="ew", bufs=3) as ew,
            tc.tile_pool(name="psl", bufs=2, space="PSUM") as psl,
            tc.tile_pool(name="psg", bufs=2, space="PSUM") as psg,
            tc.tile_pool(name="psd", bufs=2, space="PSUM") as psd,
        ):
            # ---- load weights / constants into SBUF (single strided DMAs) --
            # LSTM(0) dependencies first: the prologue AllGather is gated
            # on them, so the big GRU weights load behind them.
            wih_sb = cpool.tile([E + 1, GL], BF)
            nc.sync.dma_start(wih_sb[:, :], wih_in[:, :])
            xs_sb = cpool.tile([E + 1, ts * B], BF)
            nc.sync.dma_start(xs_sb[:, :], xs_in[:, :])
            feat_blk = cpool.tile([128, 64], F32)
            nc.sync.dma_start(feat_blk[:, :], featblk_in[:, :])
            hT_init = cpool.tile([128, KT * B], BF)
            nc.sync.dma_start(
                hT_init[:, :].rearrange("r (k b) -> r k b", k=KT),
                featT_in[:, :].rearrange("(k r) b -> r k b", k=KT))
            ones_sb = cpool.tile([1, B], BF)
            nc.sync.dma_start(ones_sb[:, :], ones_in[:, :])
            wl_sb = cpool.tile([128, KT * GL], BF)
            nc.sync.dma_start(
                wl_sb[:, :].rearrange("r (k c) -> r k c", k=KT),
                wl_in[:, :].rearrange("(k r) c -> r k c", k=KT))
            bgi_sb = cpool.tile([128, GG // 4], BF)
            nc.sync.dma_start(bgi_sb[:, :], bgi_in[:, :])
            bgh_sb = cpool.tile([1, GG], BF)
            nc.sync.dma_start(bgh_sb[:, :], bgh_in[:, :])
            wgi_sb = cpool.tile([128, KT * GG], BF)
            nc.sync.dma_start(
                wgi_sb[:, :].rearrange("r (k c) -> r k c", k=KT),
                wgi_in[:, :].rearrange("(k r) c -> r k c", k=KT))
            wgh_sb = cpool.tile([128, KT * GG], BF)
            nc.sync.dma_start(
                wgh_sb[:, :].rearrange("r (k c) -> r k c", k=KT),
                wgh_in[:, :].rearrange("(k r) c -> r k c", k=KT))
            linw_sb = cpool.tile([128, KT * VS], BF)
            nc.scalar.dma_start(
                linw_sb[:, :].rearrange("r (k c) -> r k c", k=KT),
                linw_in[:, :].rearrange("(k r) c -> r k c", k=KT))
            linb_sb = cpool.tile([VTW, VT], F32)
            nc.scalar.dma_start(linb_sb[:, :], linb_in[:, :])
            # double-buffered gathered-hg history (h-major, bf16)
            hists = [cpool.tile([128, KT * NHIST * B], BF, name=f"hist{p}")
                     for p in range(2)]

            def hist_slot_half(m, half):
                """DMA-dst AP (r, k8, b) for hg(m)'s history slot, k%2==half."""
                h = hists[(m // NHIST) % 2]
                return h[:, :].rearrange(
                    "r (k8 k2 t b) -> k2 t r k8 b", k8=NC, k2=2,
                    t=NHIST)[half][m % NHIST]

            def hist_slot_k(m, k):
                """Stationary AP [128, B] for hg(m) k-tile k."""
                h = hists[(m // NHIST) % 2]
                return h[:, k * NHIST * B + (m % NHIST) * B:
                         k * NHIST * B + (m % NHIST) * B + B]

            def emit_lstm(step, hT_ap, c_prev):
                gsum = psl.tile([128, 256], F32, tag="ps_l", name=f"ps_l{step}")
                # input-side matmuls first: xs is ready ahead of time, so
                # these run while the h-gather is still in flight.
                for j in range(4):
                    nc.tensor.matmul(
                        gsum[32 * j:32 * j + 32, :],
                        xs_sb[:, step * B:(step + 1) * B],
                        wih_sb[:, j * 256:j * 256 + 256],
                        start=True, stop=False,
                        tile_position=(0, 32 * j),
                    )
                for k in (list(range(0, KT, 2)) + list(range(1, KT, 2))):
                    for j in range(4):
                        nc.tensor.matmul(
                            gsum[32 * j:32 * j + 32, :],
                            hT_ap(k),
                            wl_sb[:, k * GL + j * 256:k * GL + j * 256 + 256],
                            start=False, stop=(k == KT - 1),
                            tile_position=(0, 32 * j),
                        )
                # gate column order is (i, f, o, g) — one fused sigmoid
                sifo = ew.tile([128, 192], F32, tag="sifo", name=f"sifo{step}")
                nc.scalar.activation(sifo[:, :], gsum[:, 0:192], AF.Sigmoid)
                tg = ew.tile([128, 64], F32, tag="tg", name=f"tg{step}")
                nc.scalar.activation(tg[:, :], gsum[:, 192:256], AF.Tanh)
                t1 = ew.tile([128, 64], F32, tag="t1", name=f"t1_{step}")
                nc.vector.tensor_mul(t1[:, :], sifo[:, 0:64], tg[:, :])
                cn = stpool.tile([128, 64], F32, tag="c_st", name=f"c_st{step}")
                nc.vector.tensor_mul(cn[:, :], sifo[:, 64:128], c_prev[:, :])
                nc.vector.tensor_add(cn[:, :], cn[:, :], t1[:, :])
                tc_ = ew.tile([128, 64], F32, tag="tc", name=f"tc{step}")
                nc.scalar.activation(tc_[:, :], cn[:, :], AF.Tanh)
                hb = ew.tile([128, 64], BF, tag="hb", name=f"hb{step}")
                nc.vector.tensor_mul(hb[:, :], sifo[:, 128:192], tc_[:, :])
                y = ew.tile([128, 128], BF, tag="y_hc", name=f"y_hc{step}")
                nc.vector.transpose(y[:, 0:64], hb[:, :])
                cb = ew.tile([128, 64], BF, tag="cb", name=f"cb{step}")
                nc.vector.tensor_copy(cb[:, :], cn[:, :])
                nc.vector.transpose(y[:, 64:128], cb[:, :])
                return cn, y

            def emit_gru(step, cT_ap, hg_prev):

                pgi = psg.tile([128, 192], F32, tag="ps_gi", name=f"ps_gi{step}")
                pgh = psg.tile([128, 192], F32, tag="ps_gh", name=f"ps_gh{step}")
                # bgh rides the gh accumulation as a ones-row matmul, emitted
                # first so it is never on the gh critical path.
                for j in range(4):
                    nc.tensor.matmul(
                        pgh[32 * j:32 * j + 32, :],
                        ones_sb[0:1, 0:B],
                        bgh_sb[0:1, j * 192:(j + 1) * 192],
                        start=True, stop=False,
                        tile_position=(0, 32 * j),
                    )
                for k in range(KT):
                    for j in range(4):
                        nc.tensor.matmul(
                            pgi[32 * j:32 * j + 32, :],
                            cT_ap(k),
                            wgi_sb[:, k * GG + j * 192:k * GG + j * 192 + 192],
                            start=(k == 0), stop=(k == KT - 1),
                            tile_position=(0, 32 * j),
                        )
                for k in (list(range(0, KT, 2)) + list(range(1, KT, 2))):
                    for j in range(4):
                        nc.tensor.matmul(
                            pgh[32 * j:32 * j + 32, :],
                            hist_slot_k(step - 1, k),
                            wgh_sb[:, k * GG + j * 192:k * GG + j * 192 + 192],
                            start=False, stop=(k == KT - 1),
                            tile_position=(0, 32 * j),
                        )
                gi_sb = ew.tile([128, 192], F32, tag="gi_sb", name=f"gi_sb{step}")
                nc.vector.tensor_add(gi_sb[:, :], pgi[:, :], bgi_sb[:, :])
                trz = ew.tile([128, 128], F32, tag="trz", name=f"trz{step}")
                nc.vector.tensor_add(trz[:, :], gi_sb[:, 0:128], pgh[:, 0:128])
                srz = ew.tile([128, 128], F32, tag="srz", name=f"srz{step}")
                nc.scalar.activation(srz[:, :], trz[:, :], AF.Sigmoid)
                x1 = ew.tile([128, 64], F32, tag="x1", name=f"x1_{step}")
                nc.vector.tensor_mul(x1[:, :], srz[:, 0:64], pgh[:, 128:192])
                nc.vector.tensor_add(x1[:, :], x1[:, :], gi_sb[:, 128:192])
                n = ew.tile([128, 64], F32, tag="n", name=f"n{step}")
                nc.scalar.activation(n[:, :], x1[:, :], AF.Tanh)
                d = ew.tile([128, 64], F32, tag="d", name=f"d{step}")
                nc.vector.tensor_sub(d[:, :], hg_prev[:, :], n[:, :])
                nc.vector.tensor_mul(d[:, :], srz[:, 64:128], d[:, :])
                hgn = stpool.tile([128, 64], F32, tag="hg_st", name=f"hg_st{step}")
                nc.vector.tensor_add(hgn[:, :], n[:, :], d[:, :])
                hgb = ew.tile([128, 64], BF, tag="hgb", name=f"hgb{step}")
                nc.vector.tensor_add(hgb[:, :], n[:, :], d[:, :])
                y_hg = ew.tile([128, 64], BF, tag="y_hg", name=f"y_hg{step}")
                nc.vector.transpose(y_hg[:, :], hgb[:, :])
                return hgn, y_hg

            def emit_bounce_out(bounce, sec, y, engine):
                # bounce[sec*HS + 128*c2 + r, b] = y[r, 32*c2 + b]
                engine.dma_start(
                    bounce[sec * HS:(sec + 1) * HS, :].rearrange(
                        "(c r) b -> r c b", c=2),
                    y[:, :].rearrange("r (c b) -> r c b", c=2))

            def emit_bounce_hc(bounce, y, engine):
                # bounce[sec*HS + 128*c2 + r, b] = y[r, sec*64 + 32*c2 + b]
                engine.dma_start(
                    bounce[:, :].rearrange("(s c r) b -> r s c b", s=2, c=2),
                    y[:, :].rearrange("r (s c b) -> r s c b", s=2, c=2))

            def emit_gather_hc(gath, hcT, engine):
                # hcT col layout (k8, t, k2, b): k8*128 + t*64 + k2*32 + b
                g5 = gath.ap().rearrange(
                    "(rank t half r) b -> half r rank t b", rank=NC, t=2,
                    half=2)
                d5 = hcT[:, :].rearrange(
                    "r (k8 t k2 b) -> k2 r k8 t b", t=2, k8=NC, k2=2)
                for half in (0, 1):
                    engine.dma_start(d5[half], g5[half])

            def hcT_h(hcT, k):
                o = (k // 2) * 128 + (k % 2) * 32
                return hcT[:, o:o + 32]

            def hcT_c(hcT, k):
                o = (k // 2) * 128 + 64 + (k % 2) * 32
                return hcT[:, o:o + 32]

            def emit_gather_in(gath, nsec, sec, dst2, engine):
                g5 = gath.ap().rearrange(
                    "(rank s half r) b -> s half r rank b", rank=NC, s=nsec,
                    half=2)
                for half in (0, 1):
                    engine.dma_start(dst2(half), g5[sec][half])

            def emit_decode_vt(c, vt, ncols):
                h = hists[c % 2]
                pd = psd.tile([128, 256], F32, tag="ps_d", name=f"ps_d{c}_{vt}")
                for k in range(KT):
                    nc.tensor.matmul(
                        pd[0:VTW, 0:ncols],
                        linw_sb[:, k * VS + vt * VTW:k * VS + (vt + 1) * VTW],
                        h[:, k * NHIST * B:k * NHIST * B + ncols],
                        start=(k == 0), stop=(k == KT - 1),
                    )
                stg = ew.tile([128, 256], F32, tag="stg", name=f"stg{c}_{vt}")
                nc.scalar.activation(stg[0:VTW, 0:ncols], pd[0:VTW, 0:ncols],
                                     AF.Identity, bias=linb_sb[:, vt:vt + 1])
                nc.scalar.dma_start(
                    out[vt * VTW:(vt + 1) * VTW,
                        c * NHIST * B:c * NHIST * B + ncols],
                    stg[0:VTW, 0:ncols])

            ag_rg = [list(range(NC))]

            def emit_ag(bounce, gname, nrows):
                gath = nc.dram_tensor(gname, [nrows * NC, B], BF,
                                      addr_space="Shared")
                nc.gpsimd.collective_compute(
                    "AllGather", mybir.AluOpType.bypass,
                    replica_groups=ag_rg,
                    ins=[bounce.ap().opt()], outs=[gath.ap().opt()],
                )
                return gath

            # ---- prologue: LSTM(0) from features ----
            c_st, y_hc = emit_lstm(0, lambda k: hT_init[:, k * B:(k + 1) * B],
                                   feat_blk)
            hg_st = feat_blk
            bounce0 = nc.dram_tensor("bounce0", [2 * HS, B], BF)
            emit_bounce_hc(bounce0, y_hc, nc.sync)
            gaths_hc = [emit_ag(bounce0, "gathc0", 2 * HS)]
            bounceg0 = nc.dram_tensor("bounceg0", [HS, B], BF)
            nc.sync.dma_start(bounceg0[:, :], featsh_in[:, :])
            gaths_hg = [emit_ag(bounceg0, "gathg0", HS)]

            # ---- main loop (GRU lags the LSTM by one window) ----
            cT_prev = None
            for s in range(ts):
                ghc = gaths_hc[s]
                hcT = spool.tile([128, 2 * KT * B], BF, tag="hcT",
                                 name=f"hcT{s}")
                # GRU(s-1) first: its inputs (hg(s-1), c(s)) were gathered
                # a window ago, so it fills TensorE while AG_hc[s] flies.
                # Engine queues are FIFO; emitting the h-gated LSTM first
                # would block the GRU behind its wait.
                if s >= 1:
                    emit_gather_in(gaths_hg[s - 1], 1, 0,
                                   lambda half: hist_slot_half(s - 2, half),
                                   nc.sync)
                    cT_s = cT_prev
                    hg_st, y_hg = emit_gru(
                        s - 1, lambda k: hcT_c(cT_s, k), hg_st)
                    bg = nc.dram_tensor(f"bounceg{s}", [HS, B], BF)
                    emit_bounce_out(bg, 0, y_hg, nc.gpsimd)
                    gaths_hg.append(emit_ag(bg, f"bgathg{s}", HS))

                # decode emitted between the GRU and LSTM blocks: in the
                # TensorE FIFO it then fills the slot after gh (whose gate
                # opened early) instead of delaying the next window's gh.
                for vt in range(VT):
                    rem = s - NHIST - 1 - vt
                    if rem >= 0 and rem % NHIST == 0:
                        emit_decode_vt(rem // NHIST, vt, NHIST * B)

                emit_gather_hc(ghc, hcT, nc.sync)
                if s < ts - 1:
                    c_st, y_hc = emit_lstm(
                        s + 1, lambda k: hcT_h(hcT, k), c_st)
                    bounce = nc.dram_tensor(f"bounce{s + 1}", [2 * HS, B],
                                            BF)
                    emit_bounce_hc(bounce, y_hc, nc.sync)
                    gaths_hc.append(
                        emit_ag(bounce, f"bgathc{s + 1}", 2 * HS))
                cT_prev = hcT

            # ---- epilogue: GRU(ts-1), last hg gather, leftover decode ----
            s = ts
            emit_gather_in(gaths_hg[s - 1], 1, 0,
                           lambda half: hist_slot_half(s - 2, half), nc.sync)
            cT_s = cT_prev
            hg_st, y_hg = emit_gru(
                s - 1, lambda k: hcT_c(cT_s, k), hg_st)
            bg = nc.dram_tensor(f"bounceg{s}", [HS, B], BF)
            emit_bounce_out(bg, 0, y_hg, nc.gpsimd)
            gaths_hg.append(emit_ag(bg, f"bgathg{s}", HS))
            emit_gather_in(gaths_hg[s], 1, 0,
                           lambda half: hist_slot_half(s - 1, half), nc.sync)
            done = set()
            for s2 in range(ts):
                for vt in range(VT):
                    rem = s2 - NHIST - 1 - vt
                    if rem >= 0 and rem % NHIST == 0:
                        done.add((rem // NHIST, vt))
            nchunks = (ts + NHIST - 1) // NHIST
            for c in range(nchunks):
                ncols = min(NHIST, ts - c * NHIST) * B
                for vt in range(VT):
                    if (c, vt) not in done:
                        emit_decode_vt(c, vt, ncols)

    nc.compile()
    return nc


def _gate_rows(core, ngates):
    """Global weight-row indices for this core's gate shard, in column order
    (jp, kappa, c2, i) with hidden-local l = 128*c2 + 32*jp + i.
    LSTM gates are reordered (i, f, o, g) so sigmoid fuses over 192 cols."""
    jp = np.arange(4)[:, None, None, None]
    kap = np.array([0, 1, 3, 2]) if ngates == 4 else np.arange(ngates)
    kappa = kap[None, :, None, None]
    c2 = np.arange(2)[None, None, :, None]
    i = np.arange(32)[None, None, None, :]
    rows = kappa * H + core * HS + 128 * c2 + 32 * jp + i
    return rows.reshape(-1)


def _prep_inputs(features, captions, emb, lstm_Wih, lstm_bih, lstm_Whh,
                 lstm_bhh, gru_Wih, gru_bih, gru_Whh, gru_bhh, lin_W, lin_b,
                 ts=TS):
    f32 = np.float32
    features = np.asarray(features, f32)
    captions = np.asarray(captions)
    emb = np.asarray(emb, f32)
    lstm_Wih = np.asarray(lstm_Wih, f32); lstm_bih = np.asarray(lstm_bih, f32)
    lstm_Whh = np.asarray(lstm_Whh, f32); lstm_bhh = np.asarray(lstm_bhh, f32)
    gru_Wih = np.asarray(gru_Wih, f32); gru_bih = np.asarray(gru_bih, f32)
    gru_Whh = np.asarray(gru_Whh, f32); gru_bhh = np.asarray(gru_bhh, f32)
    lin_W = np.asarray(lin_W, f32); lin_b = np.asarray(lin_b, f32)

    xs = emb[captions[:, :ts]]                      # [B, ts, E]
    xs_aug = np.ones((E + 1, ts * B), f32)
    xs_aug[:E, :] = xs.transpose(2, 1, 0).reshape(E, ts * B)

    featT = features.T.copy()                       # [H, B]
    ones = np.ones((1, B), f32)

    in_maps = []
    for core in range(NC):
        rl = _gate_rows(core, 4)
        rg = _gate_rows(core, 3)
        wl = lstm_Whh[rl, :].T
        wih = np.concatenate(
            [lstm_Wih[rl, :].T,
             (lstm_bih[rl] + lstm_bhh[rl])[None, :]], axis=0)
        wgi = gru_Wih[rg, :].T
        bgi = np.repeat(gru_bih[rg].reshape(4, 1, 192), B, axis=1).reshape(128, 192)
        wgh = gru_Whh[rg, :].T
        bgh = gru_bhh[rg].reshape(1, 3 * HS)
        linw = lin_W[core * VS:(core + 1) * VS, :].T
        linb = lin_b[core * VS:(core + 1) * VS].reshape(VT, VTW).T.copy()
        feat_sh = features[:, core * HS:(core + 1) * HS].T.copy()
        # feat_blk [32*jp+b, 32*c2+i] = features[b, core*HS + 128*c2+32*jp+i]
        fb = features[:, core * HS:(core + 1) * HS].reshape(B, 2, 4, 32)
        feat_blk = fb.transpose(2, 0, 1, 3).reshape(128, 64).copy()

        bf = BF16
        in_maps.append({
            "wl": wl.astype(bf), "wih": wih.astype(bf),
            "wgi": wgi.astype(bf), "bgi": bgi.astype(bf),
            "wgh": wgh.astype(bf), "bgh": bgh.astype(bf),
            "linw": linw.astype(bf),
            "linb": linb.astype(f32),
            "xs_aug": xs_aug.astype(bf),
            "featT": featT.astype(bf),
            "feat_sh": feat_sh.astype(bf),
            "feat_blk": feat_blk.astype(f32),
            "ones": ones.astype(bf),
        })
    return in_maps


def kernel(**inputs):
    ts = TS
    if ts not in _BUILD_CACHE:
        _BUILD_CACHE[ts] = _build(ts)
    nc = _BUILD_CACHE[ts]
    in_maps = _prep_inputs(**inputs, ts=ts)
    res = bass_utils.run_bass_kernel_spmd(nc, in_maps,
                                          core_ids=list(range(NC)))
    full = np.empty((B, ts, V), np.float32)
    for core in range(NC):
        o = res.results[core]["out"]                 # [VS, ts*B]
        full[:, :, core * VS:(core + 1) * VS] = (
            o.reshape(VS, ts, B).transpose(2, 1, 0))
    return full

